# revision 11
# baseline (speedup 1.0000x reference)
"""Trainium2 Bass kernel for the 21x21 correlation (cost volume) module.

Math: out[b, di*21+dj, i, j] = sum_c x1p[b, c, i+di, j+dj] * x2[b, c, i, j]
where x1p is x1 zero-padded by 10 on both spatial dims, di,dj in [0,21).

Strategy (8 NeuronCores, SPMD, no collectives):
  - Shard: batch (4) x W-halves (2). Core k -> (b = k//2, rows i in
    [64*(k%2), 64*(k%2)+64)). x1 shipped with a 10-row halo and +-10
    column padding, zero-filled on the host.
  - On-core: channels C=128 live on the SBUF partition dim (= matmul
    contraction K). For each 8x16 pixel patch, one 128-wide stationary
    operand (the x2 pixels) is multiplied against the streamed 28x36
    window of x1 positions (two matmuls of N=504), producing the
    all-pairs patch product PSUM[pixel, position]. That is copied to
    SBUF (ScalarE + VectorE in parallel) and DMA'd out as a dense
    [128, 1008] block per patch.
  - The band extraction (pixel-relative displacement gather) is a pure
    shear, which no uniform access pattern on the compute engines can
    express; it is done for free on the host with as_strided over the
    gathered [8, 8, 128, 1008] per-core output.
"""
import sys

if "/opt/trn_rl_repo" not in sys.path:
    sys.path.insert(0, "/opt/trn_rl_repo")

import numpy as np
from numpy.lib.stride_tricks import as_strided

import concourse.bass as bass
import concourse.mybir as mybir
import concourse.tile as tile
from concourse import bacc
from concourse.bass_utils import run_bass_kernel_spmd

B, C, W, H = 4, 128, 128, 128
DW = 21          # displacement window (per axis)
PAD = 10
N_CORES = 8
IB, JB = 8, 8            # patch grid per core (8 i-blocks x 8 j-blocks)
PI, PJ = 8, 16           # patch shape (pixels)
RW, QW = PI + DW - 1, PJ + DW - 1    # streamed window 28 x 36
NSTREAM = RW * QW        # 1008
HALO_ROWS = 64 + 2 * PAD     # 84
PADDED_COLS = H + 2 * PAD    # 148

# Matmul input dtype: float32r = full-rate reduced-precision fp32 matmul
# (fp32 storage). Flip to mybir.dt.float32 for exact-but-4x-slower.
MM_DT = mybir.dt.float32

_CACHE = {}


def _build_program():
    nc = bacc.Bacc("TRN2", target_bir_lowering=False, debug=False,
                   num_devices=N_CORES)
    x1h = nc.dram_tensor("x1h", [C, HALO_ROWS, PADDED_COLS], mybir.dt.float32,
                         kind="ExternalInput")
    # x2 shipped patch-major: [c, ib, jb, pi*pj] so each patch's stationary
    # operand is a single contiguous 128-element free run.
    x2s = nc.dram_tensor("x2s", [C, IB, JB, PI * PJ], mybir.dt.float32,
                         kind="ExternalInput")
    outp = nc.dram_tensor("outp", [IB, JB, 128, NSTREAM], mybir.dt.float32,
                          kind="ExternalOutput")

    with tile.TileContext(nc) as tc:
        with (
            tc.tile_pool(name="singles", bufs=1) as singles,
            tc.tile_pool(name="outs", bufs=4) as outs,
            tc.tile_pool(name="repack", bufs=3) as repack,
            tc.tile_pool(name="psum", bufs=4, space="PSUM") as psum,
        ):
            x1_sb = singles.tile([C, HALO_ROWS, PADDED_COLS], mybir.dt.float32)
            x2_sb = singles.tile([C, IB, JB, PI * PJ], mybir.dt.float32)
            nc.sync.dma_start(out=x1_sb, in_=x1h[:, :, :])
            nc.sync.dma_start(out=x2_sb, in_=x2s[:, :, :, :])

            def mm_cast(ap):
                return ap if MM_DT == mybir.dt.float32 else ap.bitcast(MM_DT)

            for ib in range(IB):
                for jb in range(JB):
                    lhsT = mm_cast(x2_sb[:, ib, jb, :])
                    ps0 = psum.tile([128, 504], mybir.dt.float32, name="ps0")
                    ps1 = psum.tile([128, 504], mybir.dt.float32, name="ps1")
                    # Repack the strided 28x36 x1 window into a contiguous
                    # run so the matmul rhs has a single free dim.
                    rp = repack.tile([128, RW, QW], mybir.dt.float32)
                    nc.scalar.copy(out=rp,
                                   in_=x1_sb[:, ib * PI:ib * PI + RW,
                                             jb * PJ:jb * PJ + QW])
                    rpf = rp.rearrange("p a b -> p (a b)")
                    nc.tensor.matmul(ps0, lhsT=lhsT,
                                     rhs=mm_cast(rpf[:, 0:504]),
                                     start=True, stop=True)
                    nc.tensor.matmul(ps1, lhsT=lhsT,
                                     rhs=mm_cast(rpf[:, 504:NSTREAM]),
                                     start=True, stop=True)
                    ot = outs.tile([128, NSTREAM], mybir.dt.float32)
                    nc.vector.tensor_copy(ot[:, 0:504], ps0)
                    nc.vector.tensor_copy(ot[:, 504:NSTREAM], ps1)
                    nc.sync.dma_start(out=outp[ib, jb], in_=ot)

    nc.finalize()
    return nc


def _shard_inputs(x1, x2):
    in_maps = []
    for k in range(N_CORES):
        b, half = divmod(k, 2)
        i0 = 64 * half
        x2sh = np.ascontiguousarray(
            x2[b][:, i0:i0 + 64, :]
            .reshape(C, IB, PI, JB, PJ)
            .transpose(0, 1, 3, 2, 4)
            .reshape(C, IB, JB, PI * PJ)
        )
        x1sh = np.zeros((C, HALO_ROWS, PADDED_COLS), np.float32)
        rlo, rhi = i0 - PAD, i0 + 64 + PAD
        slo, shi = max(rlo, 0), min(rhi, W)
        x1sh[:, slo - rlo:shi - rlo, PAD:PAD + H] = x1[b][:, slo:shi, :]
        in_maps.append({"x1h": x1sh, "x2s": x2sh})
    return in_maps


def _gather(results):
    out = np.empty((B, DW * DW, W, H), np.float32)
    for k in range(N_CORES):
        b, half = divmod(k, 2)
        i0 = 64 * half
        O = np.ascontiguousarray(results[k]["outp"])  # [8, 8, 128, 1008]
        e = O.itemsize
        s = O.strides
        sv = as_strided(
            O,
            shape=(IB, PI, JB, PJ, DW, DW),
            strides=(s[0], PJ * NSTREAM * e + QW * e, s[1],
                     NSTREAM * e + e, QW * e, e),
        )
        out[b, :, i0:i0 + 64, :] = (
            sv.transpose(4, 5, 0, 1, 2, 3).reshape(DW * DW, 64, H)
        )
    return out


def kernel(x1, x2):
    x1 = np.asarray(x1, dtype=np.float32)
    x2 = np.asarray(x2, dtype=np.float32)
    if "nc" not in _CACHE:
        _CACHE["nc"] = _build_program()
    nc = _CACHE["nc"]
    in_maps = _shard_inputs(x1, x2)
    res = run_bass_kernel_spmd(nc, in_maps, list(range(N_CORES)))
    return _gather(res.results)


# revision 23
# speedup vs baseline: 601.6317x; 601.6317x over previous
"""Trainium2 Bass kernel for the 21x21 correlation (cost volume) module.

Math: out[b, di*21+dj, i, j] = sum_c x1p[b, c, i+di, j+dj] * x2[b, c, i, j]
where x1p is x1 zero-padded by 10 on both spatial dims, di,dj in [0,21).

Strategy (8 NeuronCores, SPMD, no collectives):
  - Shard: batch (4) x W-halves (2). Core k -> (b = k//2, rows i in
    [64*(k%2), 64*(k%2)+64)). x1 shipped with a 10-row halo and +-10
    column padding, zero-filled on the host.
  - On-core: channels C=128 live on the SBUF partition dim (= matmul
    contraction K). For each 8x16 pixel patch, one 128-wide stationary
    operand (the x2 pixels) is multiplied against the streamed 28x36
    window of x1 positions (two matmuls of N=504), producing the
    all-pairs patch product PSUM[pixel, position]. That is copied to
    SBUF (ScalarE + VectorE in parallel) and DMA'd out as a dense
    [128, 1008] block per patch.
  - The band extraction (pixel-relative displacement gather) is a pure
    shear, which no uniform access pattern on the compute engines can
    express; it is done for free on the host with as_strided over the
    gathered [8, 8, 128, 1008] per-core output.
"""
import sys

if "/opt/trn_rl_repo" not in sys.path:
    sys.path.insert(0, "/opt/trn_rl_repo")

import numpy as np
from numpy.lib.stride_tricks import as_strided

import concourse.bass as bass
import concourse.mybir as mybir
import concourse.tile as tile
from concourse import bacc
from concourse.bass_utils import run_bass_kernel_spmd

B, C, W, H = 4, 128, 128, 128
DW = 21          # displacement window (per axis)
PAD = 10
N_CORES = 8
IB, JB = 8, 8            # patch grid per core (8 i-blocks x 8 j-blocks)
PI, PJ = 8, 16           # patch shape (pixels)
RW, QW = PI + DW - 1, PJ + DW - 1    # streamed window 28 x 36
NSTREAM = RW * QW        # 1008
HALO_ROWS = 64 + 2 * PAD     # 84
PADDED_COLS = H + 2 * PAD    # 148

# Matmul input dtype: float32r = full-rate reduced-precision fp32 matmul
# (fp32 storage). Flip to mybir.dt.float32 for exact-but-4x-slower.
MM_DT = mybir.dt.float32r

_CACHE = {}


def _build_program():
    nc = bacc.Bacc("TRN2", target_bir_lowering=False, debug=False,
                   num_devices=N_CORES)
    x1h = nc.dram_tensor("x1h", [C, HALO_ROWS, PADDED_COLS], mybir.dt.float32,
                         kind="ExternalInput")
    # x2 shipped patch-major: [c, ib, jb, pi*pj] so each patch's stationary
    # operand is a single contiguous 128-element free run.
    x2s = nc.dram_tensor("x2s", [C, IB, JB, PI * PJ], MM_DT,
                         kind="ExternalInput")
    outp = nc.dram_tensor("outp", [IB, JB, 128, NSTREAM], mybir.dt.float32,
                          kind="ExternalOutput")

    with tile.TileContext(nc) as tc:
        with (
            tc.tile_pool(name="singles", bufs=1) as singles,
            tc.tile_pool(name="outs", bufs=6) as outs,
            tc.tile_pool(name="repack", bufs=3) as repack,
            tc.tile_pool(name="psum", bufs=4, space="PSUM") as psum,
        ):
            x1_sb = singles.tile([C, HALO_ROWS, PADDED_COLS], mybir.dt.float32)
            x2_sb = singles.tile([C, IB, JB, PI * PJ], MM_DT)
            nc.sync.dma_start(out=x1_sb, in_=x1h[:, :, :])
            nc.sync.dma_start(out=x2_sb, in_=x2s[:, :, :, :])

            for ib in range(IB):
                for jb in range(JB):
                    lhsT = x2_sb[:, ib, jb, :]
                    ps0 = psum.tile([128, 504], mybir.dt.float32, name="ps0")
                    ps1 = psum.tile([128, 504], mybir.dt.float32, name="ps1")
                    # Repack the strided 28x36 x1 window into a contiguous
                    # run so the matmul rhs has a single free dim.
                    rp = repack.tile([128, RW, QW], MM_DT)
                    nc.scalar.copy(out=rp,
                                   in_=x1_sb[:, ib * PI:ib * PI + RW,
                                             jb * PJ:jb * PJ + QW])
                    rpf = rp.rearrange("p a b -> p (a b)")
                    nc.tensor.matmul(ps0, lhsT=lhsT,
                                     rhs=rpf[:, 0:504],
                                     start=True, stop=True)
                    nc.tensor.matmul(ps1, lhsT=lhsT,
                                     rhs=rpf[:, 504:NSTREAM],
                                     start=True, stop=True)
                    ot = outs.tile([128, NSTREAM], mybir.dt.float32)
                    nc.vector.tensor_copy(ot[:, 0:504], ps0)
                    nc.vector.tensor_copy(ot[:, 504:NSTREAM], ps1)
                    nc.sync.dma_start(out=outp[ib, jb], in_=ot)

    nc.finalize()
    return nc


def _shard_inputs(x1, x2):
    in_maps = []
    for k in range(N_CORES):
        b, half = divmod(k, 2)
        i0 = 64 * half
        x2sh = np.ascontiguousarray(
            x2[b][:, i0:i0 + 64, :]
            .reshape(C, IB, PI, JB, PJ)
            .transpose(0, 1, 3, 2, 4)
            .reshape(C, IB, JB, PI * PJ)
        )
        x1sh = np.zeros((C, HALO_ROWS, PADDED_COLS), np.float32)
        rlo, rhi = i0 - PAD, i0 + 64 + PAD
        slo, shi = max(rlo, 0), min(rhi, W)
        x1sh[:, slo - rlo:shi - rlo, PAD:PAD + H] = x1[b][:, slo:shi, :]
        in_maps.append({"x1h": x1sh, "x2s": x2sh})
    return in_maps


def _gather(results):
    out = np.empty((B, DW * DW, W, H), np.float32)
    for k in range(N_CORES):
        b, half = divmod(k, 2)
        i0 = 64 * half
        O = np.ascontiguousarray(results[k]["outp"])  # [8, 8, 128, 1008]
        e = O.itemsize
        s = O.strides
        sv = as_strided(
            O,
            shape=(IB, PI, JB, PJ, DW, DW),
            strides=(s[0], PJ * NSTREAM * e + QW * e, s[1],
                     NSTREAM * e + e, QW * e, e),
        )
        out[b, :, i0:i0 + 64, :] = (
            sv.transpose(4, 5, 0, 1, 2, 3).reshape(DW * DW, 64, H)
        )
    return out


def kernel(x1, x2):
    x1 = np.asarray(x1, dtype=np.float32)
    x2 = np.asarray(x2, dtype=np.float32)
    if "nc" not in _CACHE:
        _CACHE["nc"] = _build_program()
    nc = _CACHE["nc"]
    in_maps = _shard_inputs(x1, x2)
    res = run_bass_kernel_spmd(nc, in_maps, list(range(N_CORES)))
    return _gather(res.results)
